# revision 5
# baseline (speedup 1.0000x reference)
"""DeepSeekMoE kernel for Trainium2, 8-core SPMD.

Strategy: data-parallel over tokens (8192 tokens -> 1024/core). Each core
computes the full gate (fp32 for exact top-2 agreement with the fp32
reference), the shared expert, and the routed experts (bf16 matmuls with
fp32 PSUM accumulation) for its token shard. Routed experts are combined
via a dense gate matrix (softmax weights masked to the top-2 via a
second-max threshold), so no data-dependent control flow is needed.

Layouts (per core, tokens on the moving/free dim for layer-1 matmuls):
  xT   [D, tok]   : lhsT=W1[d,h] tiles, rhs=xT -> hT psum [h, tok]
  hT   [h, tok]   : lhsT=hT tiles, rhs=W2[h,d] -> y psum [tok, d]
No transposes needed anywhere on device.
"""

import numpy as np
import ml_dtypes

import concourse.bass as bass
import concourse.mybir as mybir
from concourse import bacc
from concourse.tile import TileContext
from concourse.bass_utils import run_bass_kernel_spmd

AF = mybir.ActivationFunctionType
ALU = mybir.AluOpType
F32 = mybir.dt.float32
BF16 = mybir.dt.bfloat16

B, L, D, H, E, K = 4, 2048, 1024, 2048, 8, 2
NCORES = 8
T = B * L                   # 8192 total tokens
TC = T // NCORES            # 1024 tokens per core
DT = D // 128               # 8 d tiles
HT = H // 128               # 16 h tiles
TT = TC // 128              # 8 token tiles per core
TCH = 2                     # token chunks of 512 for layer-1 moving dim
TW = TC // TCH              # 512


def _mlp_pass(nc, sb_w1, sb_w2, sb_hst, ps, xtb, w1_dram, w2_dram, b1_sb, b2_sb,
              ones_bf, act_fn, consume_y):
    """One expert MLP over all TC tokens.

    w1_dram: [D, H] bf16, w2_dram: [H, D] bf16, b1_sb: [128, HT] fp32 bias,
    b2_sb: [1, D] bf16 bias row. consume_y(ttile, dchunk, psum_y) handles the
    [128, 512] fp32 output tile.
    """
    hst = []
    for h in range(HT):
        w1t = sb_w1.tile([128, D], BF16, tag="w1")
        nc.sync.dma_start(
            out=w1t[:].rearrange("p (n m) -> p n m", m=128),
            in_=w1_dram[:, h * 128:(h + 1) * 128].rearrange("(n p) m -> p n m", p=128))
        hs = sb_hst.tile([128, TC], BF16, tag="hst")
        for tch in range(TCH):
            psum_h = ps.tile([128, TW], F32, tag="ps")
            for d in range(DT):
                nc.tensor.matmul(out=psum_h[:],
                                 lhsT=w1t[:, d * 128:(d + 1) * 128],
                                 rhs=xtb[:, d * TC + tch * TW: d * TC + (tch + 1) * TW],
                                 start=(d == 0), stop=(d == DT - 1))
            nc.scalar.activation(out=hs[:, tch * TW:(tch + 1) * TW], in_=psum_h[:],
                                 func=act_fn, bias=b1_sb[:, h:h + 1])
        hst.append(hs)

    w2t = []
    for h in range(HT):
        w2 = sb_w2.tile([128, D], BF16, tag="w2")
        nc.sync.dma_start(out=w2[:], in_=w2_dram[h * 128:(h + 1) * 128, :])
        w2t.append(w2)

    for tt in range(TT):
        for dc in range(2):
            psum_y = ps.tile([128, 512], F32, tag="ps")
            for h in range(HT):
                nc.tensor.matmul(out=psum_y[:],
                                 lhsT=hst[h][:, tt * 128:(tt + 1) * 128],
                                 rhs=w2t[h][:, dc * 512:(dc + 1) * 512],
                                 start=(h == 0), stop=False)
            # += be2 broadcast over tokens (ones[1,128]^T @ be2[1,512])
            nc.tensor.matmul(out=psum_y[:], lhsT=ones_bf[:1, :],
                             rhs=b2_sb[:1, dc * 512:(dc + 1) * 512],
                             start=False, stop=True)
            consume_y(tt, dc, psum_y)


def build_kernel():
    nc = bacc.Bacc()
    xT_f = nc.dram_tensor("xT_f", [D, TC], F32, kind="ExternalInput")
    xT_b = nc.dram_tensor("xT_b", [D, TC], BF16, kind="ExternalInput")
    gw1 = nc.dram_tensor("gw1", [D, H], F32, kind="ExternalInput")
    gb1 = nc.dram_tensor("gb1", [H], F32, kind="ExternalInput")
    gw2 = nc.dram_tensor("gw2", [H, E], F32, kind="ExternalInput")
    gb2 = nc.dram_tensor("gb2", [E], F32, kind="ExternalInput")
    We1 = nc.dram_tensor("We1", [E, D, H], BF16, kind="ExternalInput")
    be1 = nc.dram_tensor("be1", [E, H], F32, kind="ExternalInput")
    We2 = nc.dram_tensor("We2", [E, H, D], BF16, kind="ExternalInput")
    be2 = nc.dram_tensor("be2", [E, D], BF16, kind="ExternalInput")
    Ws1 = nc.dram_tensor("Ws1", [D, H], BF16, kind="ExternalInput")
    bs1 = nc.dram_tensor("bs1", [H], F32, kind="ExternalInput")
    Ws2 = nc.dram_tensor("Ws2", [H, D], BF16, kind="ExternalInput")
    bs2 = nc.dram_tensor("bs2", [D], BF16, kind="ExternalInput")
    out = nc.dram_tensor("out", [TC, D], F32, kind="ExternalOutput")

    with TileContext(nc) as tc:
        with tc.tile_pool(name="sb_x", bufs=1) as sb_x, \
             tc.tile_pool(name="sb_big", bufs=1) as sb_big, \
             tc.tile_pool(name="sb_w1", bufs=4) as sb_w1, \
             tc.tile_pool(name="sb_w2", bufs=HT) as sb_w2, \
             tc.tile_pool(name="sb_hst", bufs=HT) as sb_hst, \
             tc.tile_pool(name="sb_small", bufs=1) as sb_small, \
             tc.tile_pool(name="sb_rt", bufs=2 * TT) as sb_rt, \
             tc.tile_pool(name="ps", bufs=6, space="PSUM") as ps:

            # ---- constants / biases ----
            ones_bf = sb_small.tile([1, 128], BF16, tag="ones_bf")
            nc.vector.memset(ones_bf[:], 1.0)
            ones_f = sb_small.tile([1, 128], F32, tag="ones_f")
            nc.vector.memset(ones_f[:], 1.0)
            gb1_sb = sb_small.tile([128, HT], F32, tag="gb1")
            nc.sync.dma_start(out=gb1_sb[:], in_=gb1.rearrange("(n p) -> p n", p=128))
            bs1_sb = sb_small.tile([128, HT], F32, tag="bs1")
            nc.sync.dma_start(out=bs1_sb[:], in_=bs1.rearrange("(n p) -> p n", p=128))
            be1_all = sb_small.tile([128, E * HT], F32, tag="be1")
            nc.sync.dma_start(out=be1_all[:].rearrange("p (e n) -> p e n", n=HT),
                              in_=be1.rearrange("e (n p) -> p e n", p=128))
            be1_sb = [be1_all[:, e * HT:(e + 1) * HT] for e in range(E)]
            gb2_sb = sb_small.tile([1, E], F32, tag="gb2")
            nc.sync.dma_start(out=gb2_sb[:], in_=gb2[None, :])
            bs2_sb = sb_small.tile([1, D], BF16, tag="bs2")
            nc.sync.dma_start(out=bs2_sb[:], in_=bs2[None, :])
            be2_all = sb_small.tile([1, E * D], BF16, tag="be2")
            nc.sync.dma_start(out=be2_all[:].rearrange("a (e d) -> a e d", d=D),
                              in_=be2.rearrange("e d -> e d")[None, :, :])
            be2_sb = [be2_all[:, e * D:(e + 1) * D] for e in range(E)]
            gw2_sb = sb_small.tile([128, HT * E], F32, tag="gw2")
            nc.sync.dma_start(out=gw2_sb[:].rearrange("p (n m) -> p n m", m=E),
                              in_=gw2.rearrange("(n p) m -> p n m", p=128))

            # ---- x loads ----
            xtf = sb_x.tile([128, DT * TC], F32, tag="xtf")
            for d in range(DT):
                nc.sync.dma_start(out=xtf[:, d * TC:(d + 1) * TC],
                                  in_=xT_f[d * 128:(d + 1) * 128, :])
            xtb = sb_big.tile([128, DT * TC], BF16, tag="xtb")
            for d in range(DT):
                nc.sync.dma_start(out=xtb[:, d * TC:(d + 1) * TC],
                                  in_=xT_b[d * 128:(d + 1) * 128, :])

            # ---- gate: fp32 mlp -> logits -> top-2 masked softmax gates ----
            hg = []
            for h in range(HT):
                w1t = sb_w1.tile([128, D], F32, tag="w1")
                nc.sync.dma_start(
                    out=w1t[:].rearrange("p (n m) -> p n m", m=128),
                    in_=gw1[:, h * 128:(h + 1) * 128].rearrange("(n p) m -> p n m", p=128))
                hgt = sb_hst.tile([128, TC], F32, tag="hst")
                for tch in range(TCH):
                    psum_g = ps.tile([128, TW], F32, tag="ps")
                    for d in range(DT):
                        nc.tensor.matmul(out=psum_g[:],
                                         lhsT=w1t[:, d * 128:(d + 1) * 128],
                                         rhs=xtf[:, d * TC + tch * TW: d * TC + (tch + 1) * TW],
                                         start=(d == 0), stop=(d == DT - 1))
                    nc.scalar.activation(out=hgt[:, tch * TW:(tch + 1) * TW], in_=psum_g[:],
                                         func=AF.Gelu, bias=gb1_sb[:, h:h + 1])
                hg.append(hgt)

            gates_sb = []
            for tt in range(TT):
                psum_l = ps.tile([128, E], F32, tag="ps")
                for h in range(HT):
                    nc.tensor.matmul(out=psum_l[:],
                                     lhsT=hg[h][:, tt * 128:(tt + 1) * 128],
                                     rhs=gw2_sb[:, h * E:(h + 1) * E],
                                     start=(h == 0), stop=False)
                nc.tensor.matmul(out=psum_l[:], lhsT=ones_f[:1, :],
                                 rhs=gb2_sb[:1, :], start=False, stop=True)
                # routing math on [128, 8]
                lg = sb_rt.tile([128, E], F32, tag="lg")
                nc.vector.tensor_copy(lg[:], psum_l[:])
                m1 = sb_rt.tile([128, 1], F32, tag="m1")
                nc.vector.reduce_max(m1[:], lg[:], axis=mybir.AxisListType.X)
                negm1 = sb_rt.tile([128, 1], F32, tag="negm1")
                nc.vector.tensor_scalar_mul(negm1[:], m1[:], -1.0)
                ex = sb_rt.tile([128, E], F32, tag="ex")
                nc.scalar.activation(out=ex[:], in_=lg[:], func=AF.Exp, bias=negm1[:])
                ssum = sb_rt.tile([128, 1], F32, tag="ssum")
                nc.vector.reduce_sum(ssum[:], ex[:], axis=mybir.AxisListType.X)
                rs = sb_rt.tile([128, 1], F32, tag="rs")
                nc.vector.reciprocal(rs[:], ssum[:])
                eqm = sb_rt.tile([128, E], F32, tag="eqm")
                nc.vector.tensor_scalar(eqm[:], lg[:], m1[:], None, op0=ALU.is_equal)
                lm = sb_rt.tile([128, E], F32, tag="lm")
                nc.vector.scalar_tensor_tensor(out=lm[:], in0=eqm[:], scalar=-1e30,
                                               in1=lg[:], op0=ALU.mult, op1=ALU.add)
                m2 = sb_rt.tile([128, 1], F32, tag="m2")
                nc.vector.reduce_max(m2[:], lm[:], axis=mybir.AxisListType.X)
                gmask = sb_rt.tile([128, E], F32, tag="gmask")
                nc.vector.tensor_scalar(gmask[:], lg[:], m2[:], None, op0=ALU.is_ge)
                wts = sb_rt.tile([128, E], F32, tag="wts")
                nc.vector.tensor_scalar_mul(wts[:], ex[:], rs[:])
                gates = sb_rt.tile([128, E], F32, tag="gates")
                nc.vector.tensor_mul(gates[:], wts[:], gmask[:])
                gates_sb.append(gates)

            # ---- output accumulator (reuses xtf's memory via same tag) ----
            acc = sb_x.tile([128, TT * D], F32, tag="xtf")

            # ---- shared expert: initialize acc ----
            def consume_shared(tt, dc, psum_y):
                nc.vector.tensor_copy(acc[:, tt * D + dc * 512: tt * D + (dc + 1) * 512],
                                      psum_y[:])
            _mlp_pass(nc, sb_w1, sb_w2, sb_hst, ps, xtb, Ws1, Ws2, bs1_sb, bs2_sb,
                      ones_bf, AF.Gelu_apprx_tanh, consume_shared)

            # ---- routed experts: acc += gate_e * mlp_e ----
            for e in range(E):
                def consume_routed(tt, dc, psum_y, e=e):
                    sl = slice(tt * D + dc * 512, tt * D + (dc + 1) * 512)
                    nc.vector.scalar_tensor_tensor(
                        out=acc[:, sl], in0=psum_y[:], scalar=gates_sb[tt][:, e:e + 1],
                        in1=acc[:, sl], op0=ALU.mult, op1=ALU.add)
                _mlp_pass(nc, sb_w1, sb_w2, sb_hst, ps, xtb, We1[e], We2[e],
                          be1_sb[e], be2_sb[e], ones_bf, AF.Gelu_apprx_tanh,
                          consume_routed)

            # ---- store ----
            for tt in range(TT):
                nc.sync.dma_start(out=out[tt * 128:(tt + 1) * 128, :],
                                  in_=acc[:, tt * D:(tt + 1) * D])
    nc.compile()
    return nc


_NC_CACHE = None


def kernel(**inputs):
    global _NC_CACHE
    if _NC_CACHE is None:
        _NC_CACHE = build_kernel()
    nc = _NC_CACHE

    x = np.asarray(inputs["x"], dtype=np.float32).reshape(T, D)
    bf = ml_dtypes.bfloat16
    shared = {
        "gw1": np.asarray(inputs["gw1"], np.float32),
        "gb1": np.asarray(inputs["gb1"], np.float32),
        "gw2": np.asarray(inputs["gw2"], np.float32),
        "gb2": np.asarray(inputs["gb2"], np.float32),
        "We1": np.asarray(inputs["We1"], np.float32).astype(bf),
        "be1": np.asarray(inputs["be1"], np.float32),
        "We2": np.asarray(inputs["We2"], np.float32).astype(bf),
        "be2": np.asarray(inputs["be2"], np.float32).astype(bf),
        "Ws1": np.asarray(inputs["Ws1"], np.float32).astype(bf),
        "bs1": np.asarray(inputs["bs1"], np.float32),
        "Ws2": np.asarray(inputs["Ws2"], np.float32).astype(bf),
        "bs2": np.asarray(inputs["bs2"], np.float32).astype(bf),
    }
    in_maps = []
    for c in range(NCORES):
        xc = x[c * TC:(c + 1) * TC]
        xT = np.ascontiguousarray(xc.T)
        m = dict(shared)
        m["xT_f"] = xT
        m["xT_b"] = xT.astype(bf)
        in_maps.append(m)

    res = run_bass_kernel_spmd(nc, in_maps, core_ids=list(range(NCORES)))
    outc = [res.results[c]["out"] for c in range(NCORES)]
    return np.concatenate(outc, axis=0).reshape(B, L, D)


if __name__ == "__main__":
    inp = dict(np.load("/root/problem/inputs_cache.npz"))
    o = kernel(**inp)
    print("kernel out:", o.shape, o.dtype, float(np.abs(o).max()))
